# revision 25
# baseline (speedup 1.0000x reference)
"""Trainium2 Bass kernel for nn_CentralityEncoding (8 NeuronCores, SPMD).

Math (reference):
    out = x + z_in[min(in_deg,511)] + z_out[min(out_deg,511)]
        + sigmoid(cent @ W_cent + b_cent) + sigmoid(vor @ W_vor + b_vor)
        + segment_sum(edge_attr @ W_sum + b_sum, src)

Reformulations:
  * segment_sum(edge_attr @ W_sum + b_sum, src)
        = segment_sum(edge_attr, src) @ W_sum + out_deg_raw[:,None] * b_sum
    (cuts the big matmul from E=1.6M rows to N=50k rows)
  * z_in[deg] = onehot(deg) @ z_in — a K=n_bins matmul accumulated into the
    same PSUM tile as the projection.
  * sigmoid(cent*W + 0) = one ACT op with per-partition scale operand.

Sharding/layout (v2 — constant-selection):
  Nodes are sorted by out-degree (desc) and grouped into 784 chunks of 64;
  chunk ranks are snake-assigned to the 8 cores so every core gets one
  chunk per rank-octet and the same per-local-chunk tile budget
  T_k = ceil(max_deg(octet k)/2).  Within a chunk, node q's edges occupy
  partitions {q, q+64} of tiles 0..ceil(deg/2)-1, so the segment-sum
  selection matrix sel0[p, q] = (p % 64 == q) is a COMPILE-TIME CONSTANT:
  no per-tile srel compare on DVE at all.  Degree sorting makes slot
  padding ~2-3% (vs 13% for fixed-capacity buckets).

  Edge features ship as fp8 e4m3 with sigma-delta (error-feedback)
  encoding per (node, feature) stream: q_i = fp8(a_i + r_{i-1}), so the
  device's exact f32 PSUM sum telescopes to segment_sum(a) minus one
  final fp8 residual per node (~sqrt(deg)x less error than independent
  rounding; scale folds into W_sum).  Phase 2 projects agg @ W_sum' and
  adds z/sigmoid/x terms; x is resident in SBUF (one contiguous DMA) and
  out is staged in SBUF, stored in a few batched DMAs.
"""

import numpy as np
import ml_dtypes

import bass_rust
import concourse.bass as bass
import concourse.mybir as mybir
import concourse.tile as tile
from concourse.bass_utils import run_bass_kernel_spmd
from concourse.vector_clock import ScopedClock

# ----------------------------------------------------------------------------
# Problem constants (hardcoded per the harness contract).
N_NODES = 50000
N_EDGES = 1600000
NODE_DIM = 256
EDGE_DIM = 128
MAX_DEG = 512  # z tables are [512, 256]; degree clamp is 511
N_CORES = 8
P = 128
NW = 64                        # nodes per chunk
CH = 98                        # chunks per core
N_CHUNKS = N_CORES * CH        # 784
NPAIR = CH // 2                # 49 pairs of chunks -> 128-node groups
NPAD = CH * NW                 # 6272 node slots per core
NRANK = N_CHUNKS * NW          # 50176 node ranks incl. dummies
F32 = mybir.dt.float32
BF16 = mybir.dt.bfloat16
FP8 = mybir.dt.float8e4
FP8NP = mybir.dt.np(FP8)       # ml_dtypes.float8_e4m3


# ----------------------------------------------------------------------------
# Workarounds for this container's walrus build, which rejects any
# instruction carrying more than ONE semaphore wait ("Too many sync wait
# commands", CoreV3GenImpl setupSyncWait).

# (1) General: split multi-wait instructions during Tile lowering — spill
# all but the last wait onto single-wait NoOps committed just before the
# instruction on the same engine.
_orig_commit = tile.TileContext._commit_instruction


def _commit_split_waits(self, inst, lazy_reg_writes=True):
    si = getattr(inst, "sync_info", None)
    if si is not None and si.on_wait and len(si.on_wait) > 1:
        waits = list(si.on_wait)
        for w in waits[:-1]:
            nop = mybir.InstNoOp(
                name=self.nc.get_next_instruction_name(),
                sync_info=mybir.SyncInfo(on_wait=[w], on_update=[]),
                bass_nofuse=True,
                engine=inst.engine,
            )
            _orig_commit(self, nop, lazy_reg_writes)
        inst.sync_info = mybir.SyncInfo(
            on_wait=[waits[-1]], on_update=list(si.on_update)
        )
    return _orig_commit(self, inst, lazy_reg_writes)


tile.TileContext._commit_instruction = _commit_split_waits


# (2) Kernel-tail Drain: waits are attached after commit; re-emit them
# one-per-NoOp before an empty Drain.
def _patched_drain_and_barrier(self, tick_clock, wait_clock):
    nc = self.nc
    collector = nc.sync.nop(nofuse=True)
    wait_clock.add_sem_waits(
        collector.ins, ScopedClock({None: tick_clock.global_clock})
    )
    si = collector.ins.sync_info
    waits = list(si.on_wait) if si is not None else []
    if waits:
        collector.ins.sync_info = bass_rust.SyncInfo(
            on_wait=[waits[0]], on_update=[]
        )
        for w in waits[1:]:
            nop = nc.sync.nop(nofuse=True)
            nop.ins.sync_info = bass_rust.SyncInfo(on_wait=[w], on_update=[])
    nc.sync.drain()
    nc.all_engine_barrier()
    assert self.sems is not None
    popped = nc._tile_sem_poison_stack.pop()
    assert popped is self._sem_poison
    nc.clear_and_free_semaphores(list(self.sems.allocated().values()))
    nc.all_engine_barrier()


tile.TileContext._drain_and_barrier = _patched_drain_and_barrier


# ----------------------------------------------------------------------------
def build_program(T: tuple, n_groups: int, has_bsum: bool, has_bcent: bool,
                  has_bvor: bool) -> bass.Bass:
    T = list(T)
    tb = np.concatenate([[0], np.cumsum(T)]).astype(int)  # chunk -> tile base
    TOT = int(tb[-1])
    WMAX = T[0] + T[1]
    nc = bass.Bass()

    # one contiguous DRAM tensor per QUAD (4 pairs): long descriptor rows
    NQ = (NPAIR + 3) // 4
    qlo = [min(8 * q, 2 * NPAIR) for q in range(NQ)]
    qhi = [min(8 * q + 8, 2 * NPAIR) for q in range(NQ)]
    a_ds = [
        nc.declare_dram_parameter(
            f"a{q}", [P, int(tb[qhi[q]] - tb[qlo[q]]) * P], FP8,
            isOutput=False)
        for q in range(NQ)
    ]
    WQMAX = int(tb[qhi[0]] - tb[qlo[0]])
    sel0_d = nc.declare_dram_parameter("sel0", [P, NW], FP8, isOutput=False)
    x_d = nc.declare_dram_parameter("x", [P, NPAIR * NODE_DIM], BF16, isOutput=False)
    idegr_d = nc.declare_dram_parameter("idegrow", [n_groups, NPAD], F32, isOutput=False)
    odegr_d = nc.declare_dram_parameter("odegrow", [n_groups, NPAD], F32, isOutput=False)
    odegraw_d = nc.declare_dram_parameter("odegraw", [P, NPAIR], F32, isOutput=False)
    cent_d = nc.declare_dram_parameter("cent", [P, NPAIR], F32, isOutput=False)
    vor_d = nc.declare_dram_parameter("vor", [P, NPAIR], F32, isOutput=False)
    zin_d = nc.declare_dram_parameter("z_in", [MAX_DEG, NODE_DIM], F32, isOutput=False)
    zout_d = nc.declare_dram_parameter("z_out", [MAX_DEG, NODE_DIM], F32, isOutput=False)
    wsum_d = nc.declare_dram_parameter("W_sum", [EDGE_DIM, NODE_DIM], F32, isOutput=False)
    bsum_d = nc.declare_dram_parameter("b_sum", [1, NODE_DIM], F32, isOutput=False)
    wcent_d = nc.declare_dram_parameter("W_cent", [1, NODE_DIM], F32, isOutput=False)
    bcent_d = nc.declare_dram_parameter("b_cent", [1, NODE_DIM], F32, isOutput=False)
    wvor_d = nc.declare_dram_parameter("W_vor", [1, NODE_DIM], F32, isOutput=False)
    bvor_d = nc.declare_dram_parameter("b_vor", [1, NODE_DIM], F32, isOutput=False)
    out_d = nc.declare_dram_parameter("out", [P, NPAIR * NODE_DIM], BF16, isOutput=True)

    sig = mybir.ActivationFunctionType.Sigmoid

    with tile.TileContext(nc) as tc:
        with (
            tc.tile_pool(name="const", bufs=1) as const,
            tc.tile_pool(name="apool", bufs=2) as apool,
            tc.tile_pool(name="aggp", bufs=10) as aggp,
            tc.tile_pool(name="tp", bufs=4) as tp,
            tc.tile_pool(name="op", bufs=4) as op,
            tc.tile_pool(name="psp", bufs=6, space="PSUM") as psp,
            tc.tile_pool(name="prp", bufs=2, space="PSUM") as prp,
        ):
            # --- one-time constants -------------------------------------
            # partition-index constant over the node axis (degree one-hots)
            iotap_b = const.tile([P, NPAD], BF16, tag="iotap_b")
            nc.gpsimd.iota(iotap_b[:], pattern=[[0, NPAD]], base=0,
                           channel_multiplier=1,
                           allow_small_or_imprecise_dtypes=True)

            sel0_b = const.tile([P, NW], FP8, tag="sel0_b")
            nc.scalar.dma_start(out=sel0_b[:], in_=sel0_d[:])

            # whole x resident (one big contiguous DMA); whole out staged
            x_sb = const.tile([P, NPAIR * NODE_DIM], BF16, tag="x_sb")
            nc.scalar.dma_start(out=x_sb[:], in_=x_d[:])
            o_sb = const.tile([P, NPAIR * NODE_DIM], BF16, tag="o_sb")

            wsum_b = const.tile([EDGE_DIM, NODE_DIM], BF16, tag="wsum_b")
            nc.gpsimd.dma_start(out=wsum_b[:], in_=wsum_d[:])

            # z tables, one [<=128, 256] bf16 slab per degree group
            zin_sb, zout_sb, ideg_oh, odeg_oh = [], [], [], []
            for g in range(n_groups):
                k = min(MAX_DEG - g * P, P)
                zi = const.tile([P, NODE_DIM], BF16, tag=f"zin{g}")
                nc.gpsimd.dma_start(out=zi[:k, :], in_=zin_d[g * P:g * P + k, :])
                zin_sb.append((zi, k))
                zo = const.tile([P, NODE_DIM], BF16, tag=f"zout{g}")
                nc.gpsimd.dma_start(out=zo[:k, :], in_=zout_d[g * P:g * P + k, :])
                zout_sb.append((zo, k))
                # degree rows (value - 128g, offset applied on host),
                # replicated across partitions by the DMA, then one-hot
                # against the partition index
                for name, srcp, acc in (("i", idegr_d, ideg_oh),
                                        ("o", odegr_d, odeg_oh)):
                    db = const.tile([P, NPAD], BF16, tag=f"degb_{name}{g}")
                    nc.gpsimd.dma_start(
                        out=db[:],
                        in_=srcp[g:g + 1, :].to_broadcast([P, NPAD]),
                    )
                    oh = const.tile([P, NPAD], BF16, tag=f"oh_{name}{g}")
                    nc.vector.tensor_tensor(out=oh[:], in0=iotap_b[:],
                                            in1=db[:],
                                            op=mybir.AluOpType.is_equal)
                    acc.append(oh)

            def bcast_row(param, tag):
                # scalar HWDGE ring keeps these off the a-load FIFO
                t = const.tile([P, NODE_DIM], F32, tag=tag)
                nc.scalar.dma_start(
                    out=t[:], in_=param[:].to_broadcast([P, NODE_DIM])
                )
                return t

            wc_b = bcast_row(wcent_d, "wc_b")
            wv_b = bcast_row(wvor_d, "wv_b")
            bs_b = bcast_row(bsum_d, "bs_b") if has_bsum else None
            bc_b = bcast_row(bcent_d, "bc_b") if has_bcent else None
            bv_b = bcast_row(bvor_d, "bv_b") if has_bvor else None

            cent_sb = const.tile([P, NPAIR], F32, tag="cent_sb")
            nc.scalar.dma_start(out=cent_sb[:], in_=cent_d[:])
            vor_sb = const.tile([P, NPAIR], F32, tag="vor_sb")
            nc.scalar.dma_start(out=vor_sb[:], in_=vor_d[:])
            if has_bsum:
                odegraw_sb = const.tile([P, NPAIR], F32, tag="odegraw_sb")
                nc.scalar.dma_start(out=odegraw_sb[:], in_=odegraw_d[:])

            # --- main pipeline: pairs of 64-node chunks -------------------
            agg_tiles: dict[int, object] = {}
            ps_tiles: dict[int, list] = {}

            aq_tiles: dict[int, object] = {}

            def phase1(cp):
                q = cp // 4
                if cp % 4 == 0:
                    # one plain HWDGE DMA per quad; the sync ring carries
                    # ONLY these so nothing stalls the stream
                    Wq = int(tb[qhi[q]] - tb[qlo[q]])
                    aq = apool.tile([P, WQMAX * P], FP8)
                    nc.sync.dma_start(out=aq[:, :Wq * P], in_=a_ds[q][:])
                    aq_tiles[q] = aq
                at = aq_tiles[q]
                ps_tiles[cp] = []
                for s in range(2):
                    k = 2 * cp + s
                    Tk = T[k]
                    off0 = int(tb[k] - tb[qlo[q]])
                    ps = psp.tile([P, NW], F32, space="PSUM")
                    ps_tiles[cp].append(ps)
                    for t in range(Tk):
                        off = (off0 + t) * P
                        nc.tensor.matmul(
                            out=ps[:],
                            lhsT=at[:, off:off + P],
                            rhs=sel0_b[:],
                            start=(t == 0),
                            stop=(t == Tk - 1),
                        )

            def phase1_cast(cp):
                # lagged so DVE never queues behind PSUM waits
                aggt = aggp.tile([P, P], BF16)
                agg_tiles[cp] = aggt
                for s, ps in enumerate(ps_tiles.pop(cp)):
                    nc.vector.tensor_copy(aggt[:, s * NW:(s + 1) * NW], ps[:])

            def phase2(cp):
                aggt = agg_tiles.pop(cp)
                # node-major accumulation in PSUM:
                #   agg^T @ W_sum' + onehot_in^T @ z_in + onehot_out^T @ z_out
                pp = prp.tile([P, NODE_DIM], F32, space="PSUM")
                nsl = slice(cp * P, (cp + 1) * P)
                nc.tensor.matmul(out=pp[:], lhsT=aggt[:], rhs=wsum_b[:],
                                 start=True, stop=False, skip_group_check=True)
                for g in range(n_groups):
                    zi, k = zin_sb[g]
                    nc.tensor.matmul(out=pp[:], lhsT=ideg_oh[g][:k, nsl],
                                     rhs=zi[:k, :], start=False, stop=False,
                                     skip_group_check=True)
                    zo, k = zout_sb[g]
                    last = g == n_groups - 1
                    nc.tensor.matmul(out=pp[:], lhsT=odeg_oh[g][:k, nsl],
                                     rhs=zo[:k, :], start=False, stop=last,
                                     skip_group_check=True)

                # sigmoid terms: one ACT op each (bias-free fast path)
                ct = tp.tile([P, NODE_DIM], F32, tag="ct")
                if has_bcent:
                    nc.vector.tensor_mul(
                        ct[:], wc_b[:],
                        cent_sb[:, cp:cp + 1].to_broadcast([P, NODE_DIM]))
                    nc.vector.tensor_add(ct[:], ct[:], bc_b[:])
                    nc.scalar.activation(out=ct[:], in_=ct[:], func=sig)
                else:
                    nc.scalar.activation(out=ct[:], in_=wc_b[:], func=sig,
                                         scale=cent_sb[:, cp:cp + 1])
                vt = tp.tile([P, NODE_DIM], F32, tag="vt")
                if has_bvor:
                    nc.vector.tensor_mul(
                        vt[:], wv_b[:],
                        vor_sb[:, cp:cp + 1].to_broadcast([P, NODE_DIM]))
                    nc.vector.tensor_add(vt[:], vt[:], bv_b[:])
                    nc.scalar.activation(out=vt[:], in_=vt[:], func=sig)
                else:
                    nc.scalar.activation(out=vt[:], in_=wv_b[:], func=sig,
                                         scale=vor_sb[:, cp:cp + 1])

                o = op.tile([P, NODE_DIM], BF16)
                # gpsimd on purpose: DVE is busier and Pool can read SBUF
                nc.gpsimd.tensor_add(o[:], ct[:], vt[:])
                osl = slice(cp * NODE_DIM, (cp + 1) * NODE_DIM)
                if has_bsum:
                    bst = tp.tile([P, NODE_DIM], F32, tag="bst")
                    nc.vector.tensor_mul(
                        bst[:], bs_b[:],
                        odegraw_sb[:, cp:cp + 1].to_broadcast([P, NODE_DIM]))
                    nc.vector.tensor_add(o[:], o[:], bst[:])
                nc.vector.tensor_add(o[:], o[:], pp[:])
                nc.vector.tensor_add(o_sb[:, osl], o[:], x_sb[:, osl])

            STCH = 10

            def phase3(cp):
                # batched store of finished column range, every STCH pairs
                if cp % STCH == STCH - 1 or cp == NPAIR - 1:
                    lo = (cp // STCH) * STCH * NODE_DIM
                    hi = (cp + 1) * NODE_DIM
                    nc.scalar.dma_start(out=out_d[:, lo:hi], in_=o_sb[:, lo:hi])

            CAST_LAG, LAG, SLAG = 2, 5, 8
            for cp in range(NPAIR + SLAG):
                if cp < NPAIR:
                    phase1(cp)
                if CAST_LAG <= cp < NPAIR + CAST_LAG:
                    phase1_cast(cp - CAST_LAG)
                if LAG <= cp < NPAIR + LAG:
                    phase2(cp - LAG)
                if cp >= SLAG:
                    phase3(cp - SLAG)

    return nc


# ----------------------------------------------------------------------------
def prepare_inputs(x, edge_index, edge_attr, voronoi_values, centralities,
                   z_in, z_out, W_cent, b_cent, W_vor, b_vor, W_sum, b_sum):
    """Host-side sharding: degree-sort nodes into 64-node chunks,
    snake-assign chunks to cores, place each node's edges into fixed
    (tile, partition) slots; quantize edge features to int8.
    Returns (in_maps, build_key, asm)."""
    src = np.asarray(edge_index[0], dtype=np.int64)
    dst = np.asarray(edge_index[1], dtype=np.int64)
    edge_attr = np.asarray(edge_attr, dtype=np.float32)
    x = np.asarray(x, dtype=np.float32)

    deg = np.bincount(src, minlength=N_NODES).astype(np.int64)
    in_deg_raw = np.bincount(dst, minlength=N_NODES).astype(np.int64)
    in_deg = np.minimum(in_deg_raw, MAX_DEG - 1).astype(np.int64)
    out_deg = np.minimum(deg, MAX_DEG - 1).astype(np.int64)
    n_groups = max(int(max(in_deg.max(), out_deg.max())) // P + 1, 1)

    # fp8 sigma-delta encoding scale; folds into W_sum
    sd = float(edge_attr[::17].std()) or 1.0
    step = sd / 32.0

    # degree-sorted chunking
    order_nodes = np.argsort(-deg, kind="stable")
    rank_of_node = np.empty(N_NODES, dtype=np.int64)
    rank_of_node[order_nodes] = np.arange(N_NODES)
    deg_rank = np.zeros(NRANK, dtype=np.int64)
    deg_rank[:N_NODES] = deg[order_nodes]

    T = np.maximum(1, (deg_rank[::8 * NW][:CH] + 1) // 2).astype(np.int64)
    tb = np.concatenate([[0], np.cumsum(T)])
    TOT = int(tb[-1])

    r_chunk = np.arange(N_CHUNKS)
    k_chunk = r_chunk // 8
    j_chunk = r_chunk % 8
    core_of_chunk = np.where(k_chunk % 2 == 0, j_chunk, 7 - j_chunk)

    # per-edge slot placement
    rho = rank_of_node[src]
    eorder = np.argsort(rho, kind="stable")
    rhos = rho[eorder]
    st_rank = np.concatenate([[0], np.cumsum(deg_rank)])[:-1]
    i_e = np.arange(N_EDGES, dtype=np.int64) - st_rank[rhos]
    rc = rhos // NW                      # chunk rank
    k_e = rc // 8                        # local chunk index (same per core)
    c_e = core_of_chunk[rc]
    p_e = (rhos % NW) + NW * (i_e % 2)
    g_e = tb[k_e] + i_e // 2
    flat = (c_e * P + p_e) * TOT + g_e

    # Sigma-delta fp8 encoding, per (node, feature) stream: q_i =
    # fp8(a_i + r_{i-1}), so the device's exact f32 sum of q telescopes to
    # segment_sum(a) minus one final fp8 residual per node — ~sqrt(deg)x
    # less error than independent rounding.  Encoding only: the device
    # still sums every edge's own vector.
    cs = (edge_attr[eorder] * (1.0 / step)).astype(np.float32)
    q8s = np.empty((N_EDGES, EDGE_DIM), dtype=FP8NP)
    resid = np.zeros((NRANK, EDGE_DIM), dtype=np.float32)
    maxd = int(deg_rank.max())
    for i in range(maxd):
        nodes_i = np.nonzero(deg_rank > i)[0]
        idx = st_rank[nodes_i] + i
        t = cs[idx] + resid[nodes_i]
        q = t.astype(FP8NP)
        q8s[idx] = q
        resid[nodes_i] = t - q.astype(np.float32)

    a_flat = np.zeros((N_CORES * P * TOT, EDGE_DIM), dtype=FP8NP)
    a_flat[flat] = q8s
    a_grid = a_flat.reshape(N_CORES, P, TOT * EDGE_DIM)
    # per-quad contiguous blocks (long sequential HBM descriptor rows)
    NQ = (NPAIR + 3) // 4
    a_quads = [
        [np.ascontiguousarray(
            a_grid[c][:, int(tb[min(8 * q, 2 * NPAIR)]) * EDGE_DIM:
                      int(tb[min(8 * q + 8, 2 * NPAIR)]) * EDGE_DIM])
         for q in range(NQ)]
        for c in range(N_CORES)
    ]

    # node-rank -> (core, row) map
    rr = np.arange(NRANK)
    rc_r = rr // NW
    row_r = (rc_r // 8) * NW + rr % NW
    c_r = core_of_chunk[rc_r]
    rank_at = np.empty((N_CORES, NPAD), dtype=np.int64)
    rank_at[c_r, row_r] = rr

    def per_core(vals, dtype=None):
        # vals [N_NODES, ...] -> [N_CORES, NPAD, ...] via rank permutation
        shp = (NRANK,) + vals.shape[1:]
        padded = np.zeros(shp, dtype=vals.dtype)
        padded[:N_NODES] = vals[order_nodes]
        outv = padded[rank_at]
        return outv.astype(dtype) if dtype is not None else outv

    sel0 = (np.arange(P)[:, None] % NW == np.arange(NW)[None, :])
    sel0 = sel0.astype(FP8NP)

    # x in partition-major layout [P, NPAIR*NODE_DIM] (one contiguous DMA)
    x_p = per_core(x, ml_dtypes.bfloat16)
    x_pm = np.ascontiguousarray(
        x_p.reshape(N_CORES, NPAIR, P, NODE_DIM).transpose(0, 2, 1, 3)
        .reshape(N_CORES, P, NPAIR * NODE_DIM))
    ideg_p = per_core(in_deg.astype(np.float32)[:, None])[..., 0]
    odeg_p = per_core(out_deg.astype(np.float32)[:, None])[..., 0]
    goff = (np.arange(n_groups, dtype=np.float32) * P)[None, :, None]
    ideg_rows = np.ascontiguousarray(ideg_p[:, None, :] - goff)
    odeg_rows = np.ascontiguousarray(odeg_p[:, None, :] - goff)
    odegraw_p = per_core(deg.astype(np.float32)[:, None])
    cent_p = per_core(np.asarray(centralities, dtype=np.float32))
    vor_p = per_core(np.asarray(voronoi_values, dtype=np.float32))

    def col_layout(a):  # [NPAD,1] -> [P, NPAIR]  (partition-major)
        return np.ascontiguousarray(a.reshape(NPAIR, P).T)

    z_in = np.asarray(z_in, dtype=np.float32)
    z_out = np.asarray(z_out, dtype=np.float32)
    W_sum_eff = np.asarray(W_sum, dtype=np.float32) * step
    row = lambda v: np.ascontiguousarray(
        np.asarray(v, dtype=np.float32).reshape(1, NODE_DIM))
    b_sum_r, b_cent_r, b_vor_r = row(b_sum), row(b_cent), row(b_vor)
    flags = (bool(np.any(b_sum_r)), bool(np.any(b_cent_r)),
             bool(np.any(b_vor_r)))

    in_maps = []
    for c in range(N_CORES):
        in_maps.append({
            **{f"a{q}": a_quads[c][q] for q in range(NQ)},
            "sel0": sel0,
            "x": x_pm[c],
            "idegrow": ideg_rows[c],
            "odegrow": odeg_rows[c],
            "odegraw": col_layout(odegraw_p[c]),
            "cent": col_layout(cent_p[c]),
            "vor": col_layout(vor_p[c]),
            "z_in": z_in,
            "z_out": z_out,
            "W_sum": W_sum_eff,
            "b_sum": b_sum_r,
            "W_cent": row(W_cent),
            "b_cent": b_cent_r,
            "W_vor": row(W_vor),
            "b_vor": b_vor_r,
        })
    asm = {"order_nodes": order_nodes, "c_r": c_r, "row_r": row_r}
    key = (tuple(int(t) for t in T), n_groups) + flags
    return in_maps, key, asm


def assemble_output(results, asm):
    """results: list of per-core 'out' arrays [P, NPAIR*NODE_DIM]."""
    outs = np.stack([
        np.asarray(results[c], dtype=np.float32)
        .reshape(P, NPAIR, NODE_DIM).transpose(1, 0, 2).reshape(NPAD, NODE_DIM)
        for c in range(N_CORES)
    ])
    out_sorted = outs[asm["c_r"], asm["row_r"]]       # [NRANK, 256]
    out_full = np.empty((N_NODES, NODE_DIM), dtype=np.float32)
    out_full[asm["order_nodes"]] = out_sorted[:N_NODES]
    return out_full


_PROGRAM_CACHE: dict[tuple, bass.Bass] = {}


def kernel(**inputs) -> np.ndarray:
    in_maps, key, asm = prepare_inputs(**inputs)
    nc = _PROGRAM_CACHE.get(key)
    if nc is None:
        nc = build_program(*key)
        _PROGRAM_CACHE[key] = nc
    res = None
    for attempt in range(3):
        try:
            res = run_bass_kernel_spmd(nc, in_maps, core_ids=list(range(N_CORES)))
            break
        except Exception:
            # axon transiently reports "accelerator device unrecoverable";
            # a clean retry succeeds
            if attempt == 2:
                raise
    return assemble_output([res.results[i]["out"] for i in range(N_CORES)], asm)


# revision 28
# speedup vs baseline: 1.0634x; 1.0634x over previous
"""Trainium2 Bass kernel for nn_CentralityEncoding (8 NeuronCores, SPMD).

Math (reference):
    out = x + z_in[min(in_deg,511)] + z_out[min(out_deg,511)]
        + sigmoid(cent @ W_cent + b_cent) + sigmoid(vor @ W_vor + b_vor)
        + segment_sum(edge_attr @ W_sum + b_sum, src)

Reformulations:
  * segment_sum(edge_attr @ W_sum + b_sum, src)
        = segment_sum(edge_attr, src) @ W_sum + out_deg_raw[:,None] * b_sum
    (cuts the big matmul from E=1.6M rows to N=50k rows)
  * z_in[deg] = onehot(deg) @ z_in — a K=n_bins matmul accumulated into the
    same PSUM tile as the projection.
  * sigmoid(cent*W + 0) = one ACT op with per-partition scale operand.

Sharding/layout (v2 — constant-selection):
  Nodes are sorted by out-degree (desc) and grouped into 784 chunks of 64;
  chunk ranks are snake-assigned to the 8 cores so every core gets one
  chunk per rank-octet and the same per-local-chunk tile budget
  T_k = ceil(max_deg(octet k)/2).  Within a chunk, node q's edges occupy
  partitions {q, q+64} of tiles 0..ceil(deg/2)-1, so the segment-sum
  selection matrix sel0[p, q] = (p % 64 == q) is a COMPILE-TIME CONSTANT:
  no per-tile srel compare on DVE at all.  Degree sorting makes slot
  padding ~2-3% (vs 13% for fixed-capacity buckets).

  Edge features ship as fp8 e4m3 with sigma-delta (error-feedback)
  encoding per (node, feature) stream: q_i = fp8(a_i + r_{i-1}), so the
  device's exact f32 PSUM sum telescopes to segment_sum(a) minus one
  final fp8 residual per node (~sqrt(deg)x less error than independent
  rounding; scale folds into W_sum).  Phase 2 projects agg @ W_sum' and
  adds z/sigmoid/x terms; x is resident in SBUF (one contiguous DMA) and
  out is staged in SBUF, stored in a few batched DMAs.
"""

import numpy as np
import ml_dtypes

import bass_rust
import concourse.bass as bass
import concourse.mybir as mybir
import concourse.tile as tile
from concourse.bass_utils import run_bass_kernel_spmd
from concourse.vector_clock import ScopedClock

# ----------------------------------------------------------------------------
# Problem constants (hardcoded per the harness contract).
N_NODES = 50000
N_EDGES = 1600000
NODE_DIM = 256
EDGE_DIM = 128
MAX_DEG = 512  # z tables are [512, 256]; degree clamp is 511
N_CORES = 8
P = 128
NW = 64                        # nodes per chunk
CH = 98                        # chunks per core
N_CHUNKS = N_CORES * CH        # 784
NPAIR = CH // 2                # 49 pairs of chunks -> 128-node groups
NPAD = CH * NW                 # 6272 node slots per core
NRANK = N_CHUNKS * NW          # 50176 node ranks incl. dummies
F32 = mybir.dt.float32
BF16 = mybir.dt.bfloat16
FP8 = mybir.dt.float8e4
FP8NP = mybir.dt.np(FP8)       # ml_dtypes.float8_e4m3


# ----------------------------------------------------------------------------
# Workarounds for this container's walrus build, which rejects any
# instruction carrying more than ONE semaphore wait ("Too many sync wait
# commands", CoreV3GenImpl setupSyncWait).

# (1) General: split multi-wait instructions during Tile lowering — spill
# all but the last wait onto single-wait NoOps committed just before the
# instruction on the same engine.
_orig_commit = tile.TileContext._commit_instruction


def _commit_split_waits(self, inst, lazy_reg_writes=True):
    si = getattr(inst, "sync_info", None)
    if si is not None and si.on_wait and len(si.on_wait) > 1:
        waits = list(si.on_wait)
        for w in waits[:-1]:
            nop = mybir.InstNoOp(
                name=self.nc.get_next_instruction_name(),
                sync_info=mybir.SyncInfo(on_wait=[w], on_update=[]),
                bass_nofuse=True,
                engine=inst.engine,
            )
            _orig_commit(self, nop, lazy_reg_writes)
        inst.sync_info = mybir.SyncInfo(
            on_wait=[waits[-1]], on_update=list(si.on_update)
        )
    return _orig_commit(self, inst, lazy_reg_writes)


tile.TileContext._commit_instruction = _commit_split_waits


# (2) Kernel-tail Drain: waits are attached after commit; re-emit them
# one-per-NoOp before an empty Drain.
def _patched_drain_and_barrier(self, tick_clock, wait_clock):
    nc = self.nc
    collector = nc.sync.nop(nofuse=True)
    wait_clock.add_sem_waits(
        collector.ins, ScopedClock({None: tick_clock.global_clock})
    )
    si = collector.ins.sync_info
    waits = list(si.on_wait) if si is not None else []
    if waits:
        collector.ins.sync_info = bass_rust.SyncInfo(
            on_wait=[waits[0]], on_update=[]
        )
        for w in waits[1:]:
            nop = nc.sync.nop(nofuse=True)
            nop.ins.sync_info = bass_rust.SyncInfo(on_wait=[w], on_update=[])
    nc.sync.drain()
    nc.all_engine_barrier()
    assert self.sems is not None
    popped = nc._tile_sem_poison_stack.pop()
    assert popped is self._sem_poison
    nc.clear_and_free_semaphores(list(self.sems.allocated().values()))
    nc.all_engine_barrier()


tile.TileContext._drain_and_barrier = _patched_drain_and_barrier


# ----------------------------------------------------------------------------
def build_program(T: tuple, n_groups: int, has_bsum: bool, has_bcent: bool,
                  has_bvor: bool) -> bass.Bass:
    T = list(T)
    tb = np.concatenate([[0], np.cumsum(T)]).astype(int)  # chunk -> tile base
    TOT = int(tb[-1])
    WMAX = T[0] + T[1]
    nc = bass.Bass()

    # one contiguous DRAM tensor per pair: sequential HBM reads
    a_ds = [
        nc.declare_dram_parameter(
            f"a{cp}", [P, int(tb[2 * cp + 2] - tb[2 * cp]) * P], FP8,
            isOutput=False)
        for cp in range(NPAIR)
    ]
    sel0_d = nc.declare_dram_parameter("sel0", [P, NW], FP8, isOutput=False)
    x_d = nc.declare_dram_parameter("x", [P, NPAIR * NODE_DIM], BF16, isOutput=False)
    ohi_d = nc.declare_dram_parameter("ohi", [n_groups * P, NPAD], FP8, isOutput=False)
    oho_d = nc.declare_dram_parameter("oho", [n_groups * P, NPAD], FP8, isOutput=False)
    odegraw_d = nc.declare_dram_parameter("odegraw", [P, NPAIR], F32, isOutput=False)
    cent_d = nc.declare_dram_parameter("cent", [P, NPAIR], F32, isOutput=False)
    vor_d = nc.declare_dram_parameter("vor", [P, NPAIR], F32, isOutput=False)
    zin_d = nc.declare_dram_parameter("z_in", [MAX_DEG, NODE_DIM], FP8, isOutput=False)
    zout_d = nc.declare_dram_parameter("z_out", [MAX_DEG, NODE_DIM], FP8, isOutput=False)
    wsum_d = nc.declare_dram_parameter("W_sum", [EDGE_DIM, NODE_DIM], F32, isOutput=False)
    bsum_d = nc.declare_dram_parameter("b_sum", [1, NODE_DIM], F32, isOutput=False)
    wcent_d = nc.declare_dram_parameter("W_cent", [1, NODE_DIM], F32, isOutput=False)
    bcent_d = nc.declare_dram_parameter("b_cent", [1, NODE_DIM], F32, isOutput=False)
    wvor_d = nc.declare_dram_parameter("W_vor", [1, NODE_DIM], F32, isOutput=False)
    bvor_d = nc.declare_dram_parameter("b_vor", [1, NODE_DIM], F32, isOutput=False)
    out_d = nc.declare_dram_parameter("out", [P, NPAIR * NODE_DIM], BF16, isOutput=True)

    sig = mybir.ActivationFunctionType.Sigmoid

    with tile.TileContext(nc) as tc:
        with (
            tc.tile_pool(name="const", bufs=1) as const,
            tc.tile_pool(name="apool", bufs=6) as apool,
            tc.tile_pool(name="aggp", bufs=10) as aggp,
            tc.tile_pool(name="tp", bufs=4) as tp,
            tc.tile_pool(name="op", bufs=4) as op,
            tc.tile_pool(name="psp", bufs=6, space="PSUM") as psp,
            tc.tile_pool(name="prp", bufs=2, space="PSUM") as prp,
        ):
            # --- one-time constants -------------------------------------
            sel0_b = const.tile([P, NW], FP8, tag="sel0_b")
            nc.scalar.dma_start(out=sel0_b[:], in_=sel0_d[:])

            # whole x resident (one big contiguous DMA); whole out staged
            x_sb = const.tile([P, NPAIR * NODE_DIM], BF16, tag="x_sb")
            nc.scalar.dma_start(out=x_sb[:], in_=x_d[:])
            o_sb = const.tile([P, NPAIR * NODE_DIM], BF16, tag="o_sb")

            wsum_b = const.tile([EDGE_DIM, NODE_DIM], BF16, tag="wsum_b")
            nc.gpsimd.dma_start(out=wsum_b[:], in_=wsum_d[:])

            # z tables (fp8) + host-computed degree one-hots (fp8)
            zin_sb, zout_sb, ideg_oh, odeg_oh = [], [], [], []
            for g in range(n_groups):
                k = min(MAX_DEG - g * P, P)
                zi = const.tile([P, NODE_DIM], FP8, tag=f"zin{g}")
                nc.gpsimd.dma_start(out=zi[:k, :], in_=zin_d[g * P:g * P + k, :])
                zin_sb.append((zi, k))
                zo = const.tile([P, NODE_DIM], FP8, tag=f"zout{g}")
                nc.gpsimd.dma_start(out=zo[:k, :], in_=zout_d[g * P:g * P + k, :])
                zout_sb.append((zo, k))
                for name, srcp, acc in (("i", ohi_d, ideg_oh),
                                        ("o", oho_d, odeg_oh)):
                    oh = const.tile([P, NPAD], FP8, tag=f"oh_{name}{g}")
                    nc.gpsimd.dma_start(out=oh[:],
                                        in_=srcp[g * P:(g + 1) * P, :])
                    acc.append(oh)

            def bcast_row(param, tag):
                # scalar HWDGE ring keeps these off the a-load FIFO
                t = const.tile([P, NODE_DIM], F32, tag=tag)
                nc.scalar.dma_start(
                    out=t[:], in_=param[:].to_broadcast([P, NODE_DIM])
                )
                return t

            wc_b = bcast_row(wcent_d, "wc_b")
            wv_b = bcast_row(wvor_d, "wv_b")
            bs_b = bcast_row(bsum_d, "bs_b") if has_bsum else None
            bc_b = bcast_row(bcent_d, "bc_b") if has_bcent else None
            bv_b = bcast_row(bvor_d, "bv_b") if has_bvor else None

            cent_sb = const.tile([P, NPAIR], F32, tag="cent_sb")
            nc.scalar.dma_start(out=cent_sb[:], in_=cent_d[:])
            vor_sb = const.tile([P, NPAIR], F32, tag="vor_sb")
            nc.scalar.dma_start(out=vor_sb[:], in_=vor_d[:])
            if has_bsum:
                odegraw_sb = const.tile([P, NPAIR], F32, tag="odegraw_sb")
                nc.scalar.dma_start(out=odegraw_sb[:], in_=odegraw_d[:])

            # --- main pipeline: pairs of 64-node chunks -------------------
            agg_tiles: dict[int, object] = {}
            ps_tiles: dict[int, list] = {}

            def phase1(cp):
                # one plain HWDGE DMA per pair; the sync ring carries ONLY
                # these so nothing stalls the stream
                W = int(tb[2 * cp + 2] - tb[2 * cp])
                at = apool.tile([P, WMAX * P], FP8)
                nc.sync.dma_start(out=at[:, :W * P], in_=a_ds[cp][:])
                ps_tiles[cp] = []
                for s in range(2):
                    k = 2 * cp + s
                    Tk = T[k]
                    off0 = int(tb[k] - tb[2 * cp])
                    ps = psp.tile([P, NW], F32, space="PSUM")
                    ps_tiles[cp].append(ps)
                    for t in range(Tk):
                        off = (off0 + t) * P
                        nc.tensor.matmul(
                            out=ps[:],
                            lhsT=at[:, off:off + P],
                            rhs=sel0_b[:],
                            start=(t == 0),
                            stop=(t == Tk - 1),
                        )

            def phase1_cast(cp):
                # lagged so DVE never queues behind PSUM waits
                aggt = aggp.tile([P, P], BF16)
                agg_tiles[cp] = aggt
                for s, ps in enumerate(ps_tiles.pop(cp)):
                    nc.vector.tensor_copy(aggt[:, s * NW:(s + 1) * NW], ps[:])

            def phase2(cp):
                aggt = agg_tiles.pop(cp)
                # node-major accumulation in PSUM:
                #   agg^T @ W_sum' + onehot_in^T @ z_in + onehot_out^T @ z_out
                pp = prp.tile([P, NODE_DIM], F32, space="PSUM")
                nsl = slice(cp * P, (cp + 1) * P)
                nc.tensor.matmul(out=pp[:], lhsT=aggt[:], rhs=wsum_b[:],
                                 start=True, stop=False, skip_group_check=True)
                for g in range(n_groups):
                    zi, k = zin_sb[g]
                    nc.tensor.matmul(out=pp[:], lhsT=ideg_oh[g][:k, nsl],
                                     rhs=zi[:k, :], start=False, stop=False,
                                     skip_group_check=True)
                    zo, k = zout_sb[g]
                    last = g == n_groups - 1
                    nc.tensor.matmul(out=pp[:], lhsT=odeg_oh[g][:k, nsl],
                                     rhs=zo[:k, :], start=False, stop=last,
                                     skip_group_check=True)

                # sigmoid terms: one ACT op each (bias-free fast path)
                ct = tp.tile([P, NODE_DIM], F32, tag="ct")
                if has_bcent:
                    nc.vector.tensor_mul(
                        ct[:], wc_b[:],
                        cent_sb[:, cp:cp + 1].to_broadcast([P, NODE_DIM]))
                    nc.vector.tensor_add(ct[:], ct[:], bc_b[:])
                    nc.scalar.activation(out=ct[:], in_=ct[:], func=sig)
                else:
                    nc.scalar.activation(out=ct[:], in_=wc_b[:], func=sig,
                                         scale=cent_sb[:, cp:cp + 1])
                vt = tp.tile([P, NODE_DIM], F32, tag="vt")
                if has_bvor:
                    nc.vector.tensor_mul(
                        vt[:], wv_b[:],
                        vor_sb[:, cp:cp + 1].to_broadcast([P, NODE_DIM]))
                    nc.vector.tensor_add(vt[:], vt[:], bv_b[:])
                    nc.scalar.activation(out=vt[:], in_=vt[:], func=sig)
                else:
                    nc.scalar.activation(out=vt[:], in_=wv_b[:], func=sig,
                                         scale=vor_sb[:, cp:cp + 1])

                o = op.tile([P, NODE_DIM], BF16)
                # gpsimd on purpose: DVE is busier and Pool can read SBUF
                nc.gpsimd.tensor_add(o[:], ct[:], vt[:])
                osl = slice(cp * NODE_DIM, (cp + 1) * NODE_DIM)
                if has_bsum:
                    bst = tp.tile([P, NODE_DIM], F32, tag="bst")
                    nc.vector.tensor_mul(
                        bst[:], bs_b[:],
                        odegraw_sb[:, cp:cp + 1].to_broadcast([P, NODE_DIM]))
                    nc.vector.tensor_add(o[:], o[:], bst[:])
                nc.vector.tensor_add(o[:], o[:], pp[:])
                nc.vector.tensor_add(o_sb[:, osl], o[:], x_sb[:, osl])

            STCH = 10

            def phase3(cp):
                # batched store of finished column range, every STCH pairs
                if cp % STCH == STCH - 1 or cp == NPAIR - 1:
                    lo = (cp // STCH) * STCH * NODE_DIM
                    hi = (cp + 1) * NODE_DIM
                    nc.scalar.dma_start(out=out_d[:, lo:hi], in_=o_sb[:, lo:hi])

            CAST_LAG, LAG, SLAG = 2, 4, 7
            for cp in range(NPAIR + SLAG):
                if cp < NPAIR:
                    phase1(cp)
                if CAST_LAG <= cp < NPAIR + CAST_LAG:
                    phase1_cast(cp - CAST_LAG)
                if LAG <= cp < NPAIR + LAG:
                    phase2(cp - LAG)
                if cp >= SLAG:
                    phase3(cp - SLAG)

    return nc


# ----------------------------------------------------------------------------
def prepare_inputs(x, edge_index, edge_attr, voronoi_values, centralities,
                   z_in, z_out, W_cent, b_cent, W_vor, b_vor, W_sum, b_sum):
    """Host-side sharding: degree-sort nodes into 64-node chunks,
    snake-assign chunks to cores, place each node's edges into fixed
    (tile, partition) slots; quantize edge features to int8.
    Returns (in_maps, build_key, asm)."""
    src = np.asarray(edge_index[0], dtype=np.int64)
    dst = np.asarray(edge_index[1], dtype=np.int64)
    edge_attr = np.asarray(edge_attr, dtype=np.float32)
    x = np.asarray(x, dtype=np.float32)

    deg = np.bincount(src, minlength=N_NODES).astype(np.int64)
    in_deg_raw = np.bincount(dst, minlength=N_NODES).astype(np.int64)
    in_deg = np.minimum(in_deg_raw, MAX_DEG - 1).astype(np.int64)
    out_deg = np.minimum(deg, MAX_DEG - 1).astype(np.int64)
    n_groups = max(int(max(in_deg.max(), out_deg.max())) // P + 1, 1)

    # fp8 sigma-delta encoding scale; folds into W_sum
    sd = float(edge_attr[::17].std()) or 1.0
    step = sd / 32.0

    # degree-sorted chunking
    order_nodes = np.argsort(-deg, kind="stable")
    rank_of_node = np.empty(N_NODES, dtype=np.int64)
    rank_of_node[order_nodes] = np.arange(N_NODES)
    deg_rank = np.zeros(NRANK, dtype=np.int64)
    deg_rank[:N_NODES] = deg[order_nodes]

    T = np.maximum(1, (deg_rank[::8 * NW][:CH] + 1) // 2).astype(np.int64)
    tb = np.concatenate([[0], np.cumsum(T)])
    TOT = int(tb[-1])

    r_chunk = np.arange(N_CHUNKS)
    k_chunk = r_chunk // 8
    j_chunk = r_chunk % 8
    core_of_chunk = np.where(k_chunk % 2 == 0, j_chunk, 7 - j_chunk)

    # per-edge slot placement
    rho = rank_of_node[src]
    eorder = np.argsort(rho, kind="stable")
    rhos = rho[eorder]
    st_rank = np.concatenate([[0], np.cumsum(deg_rank)])[:-1]
    i_e = np.arange(N_EDGES, dtype=np.int64) - st_rank[rhos]
    rc = rhos // NW                      # chunk rank
    k_e = rc // 8                        # local chunk index (same per core)
    c_e = core_of_chunk[rc]
    p_e = (rhos % NW) + NW * (i_e % 2)
    g_e = tb[k_e] + i_e // 2
    flat = (c_e * P + p_e) * TOT + g_e

    # Sigma-delta fp8 encoding, per (node, feature) stream: q_i =
    # fp8(a_i + r_{i-1}), so the device's exact f32 sum of q telescopes to
    # segment_sum(a) minus one final fp8 residual per node — ~sqrt(deg)x
    # less error than independent rounding.  Encoding only: the device
    # still sums every edge's own vector.
    cs = (edge_attr[eorder] * (1.0 / step)).astype(np.float32)
    q8s = np.empty((N_EDGES, EDGE_DIM), dtype=FP8NP)
    resid = np.zeros((NRANK, EDGE_DIM), dtype=np.float32)
    maxd = int(deg_rank.max())
    for i in range(maxd):
        nodes_i = np.nonzero(deg_rank > i)[0]
        idx = st_rank[nodes_i] + i
        t = cs[idx] + resid[nodes_i]
        q = t.astype(FP8NP)
        q8s[idx] = q
        resid[nodes_i] = t - q.astype(np.float32)

    a_flat = np.zeros((N_CORES * P * TOT, EDGE_DIM), dtype=FP8NP)
    a_flat[flat] = q8s
    a_grid = a_flat.reshape(N_CORES, P, TOT * EDGE_DIM)
    # per-pair contiguous blocks (sequential HBM reads)
    a_pairs = [
        [np.ascontiguousarray(
            a_grid[c][:, int(tb[2 * cp]) * EDGE_DIM:int(tb[2 * cp + 2]) * EDGE_DIM])
         for cp in range(NPAIR)]
        for c in range(N_CORES)
    ]

    # node-rank -> (core, row) map
    rr = np.arange(NRANK)
    rc_r = rr // NW
    row_r = (rc_r // 8) * NW + rr % NW
    c_r = core_of_chunk[rc_r]
    rank_at = np.empty((N_CORES, NPAD), dtype=np.int64)
    rank_at[c_r, row_r] = rr

    def per_core(vals, dtype=None):
        # vals [N_NODES, ...] -> [N_CORES, NPAD, ...] via rank permutation
        shp = (NRANK,) + vals.shape[1:]
        padded = np.zeros(shp, dtype=vals.dtype)
        padded[:N_NODES] = vals[order_nodes]
        outv = padded[rank_at]
        return outv.astype(dtype) if dtype is not None else outv

    sel0 = (np.arange(P)[:, None] % NW == np.arange(NW)[None, :])
    sel0 = sel0.astype(FP8NP)

    # x in partition-major layout [P, NPAIR*NODE_DIM] (one contiguous DMA)
    x_p = per_core(x, ml_dtypes.bfloat16)
    x_pm = np.ascontiguousarray(
        x_p.reshape(N_CORES, NPAIR, P, NODE_DIM).transpose(0, 2, 1, 3)
        .reshape(N_CORES, P, NPAIR * NODE_DIM))
    # host-computed degree one-hots, [n_groups*P, NPAD] fp8 per core
    bins = np.arange(n_groups * P, dtype=np.int64)
    ideg_p = per_core(in_deg[:, None])[..., 0]
    odeg_p = per_core(out_deg[:, None])[..., 0]
    ohi = (ideg_p[:, None, :] == bins[None, :, None]).astype(FP8NP)
    oho = (odeg_p[:, None, :] == bins[None, :, None]).astype(FP8NP)
    odegraw_p = per_core(deg.astype(np.float32)[:, None])
    cent_p = per_core(np.asarray(centralities, dtype=np.float32))
    vor_p = per_core(np.asarray(voronoi_values, dtype=np.float32))

    def col_layout(a):  # [NPAD,1] -> [P, NPAIR]  (partition-major)
        return np.ascontiguousarray(a.reshape(NPAIR, P).T)

    z_in = np.asarray(z_in, dtype=np.float32).astype(FP8NP)
    z_out = np.asarray(z_out, dtype=np.float32).astype(FP8NP)
    W_sum_eff = np.asarray(W_sum, dtype=np.float32) * step
    row = lambda v: np.ascontiguousarray(
        np.asarray(v, dtype=np.float32).reshape(1, NODE_DIM))
    b_sum_r, b_cent_r, b_vor_r = row(b_sum), row(b_cent), row(b_vor)
    flags = (bool(np.any(b_sum_r)), bool(np.any(b_cent_r)),
             bool(np.any(b_vor_r)))

    in_maps = []
    for c in range(N_CORES):
        in_maps.append({
            **{f"a{cp}": a_pairs[c][cp] for cp in range(NPAIR)},
            "sel0": sel0,
            "x": x_pm[c],
            "ohi": np.ascontiguousarray(ohi[c]),
            "oho": np.ascontiguousarray(oho[c]),
            "odegraw": col_layout(odegraw_p[c]),
            "cent": col_layout(cent_p[c]),
            "vor": col_layout(vor_p[c]),
            "z_in": z_in,
            "z_out": z_out,
            "W_sum": W_sum_eff,
            "b_sum": b_sum_r,
            "W_cent": row(W_cent),
            "b_cent": b_cent_r,
            "W_vor": row(W_vor),
            "b_vor": b_vor_r,
        })
    asm = {"order_nodes": order_nodes, "c_r": c_r, "row_r": row_r}
    key = (tuple(int(t) for t in T), n_groups) + flags
    return in_maps, key, asm


def assemble_output(results, asm):
    """results: list of per-core 'out' arrays [P, NPAIR*NODE_DIM]."""
    outs = np.stack([
        np.asarray(results[c], dtype=np.float32)
        .reshape(P, NPAIR, NODE_DIM).transpose(1, 0, 2).reshape(NPAD, NODE_DIM)
        for c in range(N_CORES)
    ])
    out_sorted = outs[asm["c_r"], asm["row_r"]]       # [NRANK, 256]
    out_full = np.empty((N_NODES, NODE_DIM), dtype=np.float32)
    out_full[asm["order_nodes"]] = out_sorted[:N_NODES]
    return out_full


_PROGRAM_CACHE: dict[tuple, bass.Bass] = {}


def kernel(**inputs) -> np.ndarray:
    in_maps, key, asm = prepare_inputs(**inputs)
    nc = _PROGRAM_CACHE.get(key)
    if nc is None:
        nc = build_program(*key)
        _PROGRAM_CACHE[key] = nc
    res = None
    for attempt in range(3):
        try:
            res = run_bass_kernel_spmd(nc, in_maps, core_ids=list(range(N_CORES)))
            break
        except Exception:
            # axon transiently reports "accelerator device unrecoverable";
            # a clean retry succeeds
            if attempt == 2:
                raise
    return assemble_output([res.results[i]["out"] for i in range(N_CORES)], asm)


# revision 29
# speedup vs baseline: 1.1639x; 1.0945x over previous
"""Trainium2 Bass kernel for nn_CentralityEncoding (8 NeuronCores, SPMD).

Math (reference):
    out = x + z_in[min(in_deg,511)] + z_out[min(out_deg,511)]
        + sigmoid(cent @ W_cent + b_cent) + sigmoid(vor @ W_vor + b_vor)
        + segment_sum(edge_attr @ W_sum + b_sum, src)

Reformulations:
  * segment_sum(edge_attr @ W_sum + b_sum, src)
        = segment_sum(edge_attr, src) @ W_sum + out_deg_raw[:,None] * b_sum
    (cuts the big matmul from E=1.6M rows to N=50k rows)
  * z_in[deg] = onehot(deg) @ z_in — a K=n_bins matmul accumulated into the
    same PSUM tile as the projection.
  * sigmoid(cent*W + 0) = one ACT op with per-partition scale operand.

Sharding/layout (v2 — constant-selection):
  Nodes are sorted by out-degree (desc) and grouped into 784 chunks of 64;
  chunk ranks are snake-assigned to the 8 cores so every core gets one
  chunk per rank-octet and the same per-local-chunk tile budget
  T_k = ceil(max_deg(octet k)/2).  Within a chunk, node q's edges occupy
  partitions {q, q+64} of tiles 0..ceil(deg/2)-1, so the segment-sum
  selection matrix sel0[p, q] = (p % 64 == q) is a COMPILE-TIME CONSTANT:
  no per-tile srel compare on DVE at all.  Degree sorting makes slot
  padding ~2-3% (vs 13% for fixed-capacity buckets).

  Edge features ship as fp8 e4m3 with sigma-delta (error-feedback)
  encoding per (node, feature) stream: q_i = fp8(a_i + r_{i-1}), so the
  device's exact f32 PSUM sum telescopes to segment_sum(a) minus one
  final fp8 residual per node (~sqrt(deg)x less error than independent
  rounding; scale folds into W_sum).  Phase 2 projects agg @ W_sum' and
  adds z/sigmoid/x terms; x is resident in SBUF (one contiguous DMA) and
  out is staged in SBUF, stored in a few batched DMAs.
"""

import numpy as np
import ml_dtypes

import bass_rust
import concourse.bass as bass
import concourse.mybir as mybir
import concourse.tile as tile
from concourse.bass_utils import run_bass_kernel_spmd
from concourse.vector_clock import ScopedClock

# ----------------------------------------------------------------------------
# Problem constants (hardcoded per the harness contract).
N_NODES = 50000
N_EDGES = 1600000
NODE_DIM = 256
EDGE_DIM = 128
MAX_DEG = 512  # z tables are [512, 256]; degree clamp is 511
N_CORES = 8
P = 128
NW = 64                        # nodes per chunk
CH = 98                        # chunks per core
N_CHUNKS = N_CORES * CH        # 784
NPAIR = CH // 2                # 49 pairs of chunks -> 128-node groups
NPAD = CH * NW                 # 6272 node slots per core
NRANK = N_CHUNKS * NW          # 50176 node ranks incl. dummies
F32 = mybir.dt.float32
BF16 = mybir.dt.bfloat16
FP8 = mybir.dt.float8e4
FP8NP = mybir.dt.np(FP8)       # ml_dtypes.float8_e4m3


# ----------------------------------------------------------------------------
# Workarounds for this container's walrus build, which rejects any
# instruction carrying more than ONE semaphore wait ("Too many sync wait
# commands", CoreV3GenImpl setupSyncWait).

# (1) General: split multi-wait instructions during Tile lowering — spill
# all but the last wait onto single-wait NoOps committed just before the
# instruction on the same engine.
_orig_commit = tile.TileContext._commit_instruction


def _commit_split_waits(self, inst, lazy_reg_writes=True):
    si = getattr(inst, "sync_info", None)
    if si is not None and si.on_wait and len(si.on_wait) > 1:
        waits = list(si.on_wait)
        for w in waits[:-1]:
            nop = mybir.InstNoOp(
                name=self.nc.get_next_instruction_name(),
                sync_info=mybir.SyncInfo(on_wait=[w], on_update=[]),
                bass_nofuse=True,
                engine=inst.engine,
            )
            _orig_commit(self, nop, lazy_reg_writes)
        inst.sync_info = mybir.SyncInfo(
            on_wait=[waits[-1]], on_update=list(si.on_update)
        )
    return _orig_commit(self, inst, lazy_reg_writes)


tile.TileContext._commit_instruction = _commit_split_waits


# (2) Kernel-tail Drain: waits are attached after commit; re-emit them
# one-per-NoOp before an empty Drain.
def _patched_drain_and_barrier(self, tick_clock, wait_clock):
    nc = self.nc
    collector = nc.sync.nop(nofuse=True)
    wait_clock.add_sem_waits(
        collector.ins, ScopedClock({None: tick_clock.global_clock})
    )
    si = collector.ins.sync_info
    waits = list(si.on_wait) if si is not None else []
    if waits:
        collector.ins.sync_info = bass_rust.SyncInfo(
            on_wait=[waits[0]], on_update=[]
        )
        for w in waits[1:]:
            nop = nc.sync.nop(nofuse=True)
            nop.ins.sync_info = bass_rust.SyncInfo(on_wait=[w], on_update=[])
    nc.sync.drain()
    nc.all_engine_barrier()
    assert self.sems is not None
    popped = nc._tile_sem_poison_stack.pop()
    assert popped is self._sem_poison
    nc.clear_and_free_semaphores(list(self.sems.allocated().values()))
    nc.all_engine_barrier()


tile.TileContext._drain_and_barrier = _patched_drain_and_barrier


# ----------------------------------------------------------------------------
def build_program(T: tuple, n_groups: int, has_bsum: bool, has_bcent: bool,
                  has_bvor: bool) -> bass.Bass:
    T = list(T)
    tb = np.concatenate([[0], np.cumsum(T)]).astype(int)  # chunk -> tile base
    TOT = int(tb[-1])
    WMAX = T[0] + T[1]
    nc = bass.Bass()

    # one contiguous DRAM tensor per pair: sequential HBM reads
    a_ds = [
        nc.declare_dram_parameter(
            f"a{cp}", [P, int(tb[2 * cp + 2] - tb[2 * cp]) * P], FP8,
            isOutput=False)
        for cp in range(NPAIR)
    ]
    sel0_d = nc.declare_dram_parameter("sel0", [P, NW], FP8, isOutput=False)
    x_d = nc.declare_dram_parameter("x", [P, NPAIR * NODE_DIM], FP8, isOutput=False)
    ident_d = nc.declare_dram_parameter("ident", [P, P], FP8, isOutput=False)
    ohi_d = nc.declare_dram_parameter("ohi", [n_groups * P, NPAD], FP8, isOutput=False)
    oho_d = nc.declare_dram_parameter("oho", [n_groups * P, NPAD], FP8, isOutput=False)
    odegraw_d = nc.declare_dram_parameter("odegraw", [P, NPAIR], F32, isOutput=False)
    cent_d = nc.declare_dram_parameter("cent", [P, NPAIR], F32, isOutput=False)
    vor_d = nc.declare_dram_parameter("vor", [P, NPAIR], F32, isOutput=False)
    zin_d = nc.declare_dram_parameter("z_in", [MAX_DEG, NODE_DIM], FP8, isOutput=False)
    zout_d = nc.declare_dram_parameter("z_out", [MAX_DEG, NODE_DIM], FP8, isOutput=False)
    wsum_d = nc.declare_dram_parameter("W_sum", [EDGE_DIM, NODE_DIM], F32, isOutput=False)
    bsum_d = nc.declare_dram_parameter("b_sum", [1, NODE_DIM], F32, isOutput=False)
    wcent_d = nc.declare_dram_parameter("W_cent", [1, NODE_DIM], F32, isOutput=False)
    bcent_d = nc.declare_dram_parameter("b_cent", [1, NODE_DIM], F32, isOutput=False)
    wvor_d = nc.declare_dram_parameter("W_vor", [1, NODE_DIM], F32, isOutput=False)
    bvor_d = nc.declare_dram_parameter("b_vor", [1, NODE_DIM], F32, isOutput=False)
    out_d = nc.declare_dram_parameter("out", [P, NPAIR * NODE_DIM], BF16, isOutput=True)

    sig = mybir.ActivationFunctionType.Sigmoid

    with tile.TileContext(nc) as tc:
        with (
            tc.tile_pool(name="const", bufs=1) as const,
            tc.tile_pool(name="apool", bufs=6) as apool,
            tc.tile_pool(name="aggp", bufs=10) as aggp,
            tc.tile_pool(name="tp", bufs=4) as tp,
            tc.tile_pool(name="op", bufs=4) as op,
            tc.tile_pool(name="psp", bufs=6, space="PSUM") as psp,
            tc.tile_pool(name="prp", bufs=2, space="PSUM") as prp,
        ):
            # --- one-time constants -------------------------------------
            sel0_b = const.tile([P, NW], FP8, tag="sel0_b")
            nc.scalar.dma_start(out=sel0_b[:], in_=sel0_d[:])

            # whole x resident (one big contiguous DMA); whole out staged
            x_sb = const.tile([P, NPAIR * NODE_DIM], FP8, tag="x_sb")
            nc.scalar.dma_start(out=x_sb[:], in_=x_d[:])
            ident_b = const.tile([P, P], FP8, tag="ident_b")
            nc.scalar.dma_start(out=ident_b[:], in_=ident_d[:])
            o_sb = const.tile([P, NPAIR * NODE_DIM], BF16, tag="o_sb")

            wsum_b = const.tile([EDGE_DIM, NODE_DIM], BF16, tag="wsum_b")
            nc.gpsimd.dma_start(out=wsum_b[:], in_=wsum_d[:])

            # z tables (fp8) + host-computed degree one-hots (fp8)
            zin_sb, zout_sb, ideg_oh, odeg_oh = [], [], [], []
            for g in range(n_groups):
                k = min(MAX_DEG - g * P, P)
                zi = const.tile([P, NODE_DIM], FP8, tag=f"zin{g}")
                nc.gpsimd.dma_start(out=zi[:k, :], in_=zin_d[g * P:g * P + k, :])
                zin_sb.append((zi, k))
                zo = const.tile([P, NODE_DIM], FP8, tag=f"zout{g}")
                nc.gpsimd.dma_start(out=zo[:k, :], in_=zout_d[g * P:g * P + k, :])
                zout_sb.append((zo, k))
                for name, srcp, acc in (("i", ohi_d, ideg_oh),
                                        ("o", oho_d, odeg_oh)):
                    oh = const.tile([P, NPAD], FP8, tag=f"oh_{name}{g}")
                    nc.gpsimd.dma_start(out=oh[:],
                                        in_=srcp[g * P:(g + 1) * P, :])
                    acc.append(oh)

            def bcast_row(param, tag):
                # scalar HWDGE ring keeps these off the a-load FIFO
                t = const.tile([P, NODE_DIM], F32, tag=tag)
                nc.scalar.dma_start(
                    out=t[:], in_=param[:].to_broadcast([P, NODE_DIM])
                )
                return t

            wc_b = bcast_row(wcent_d, "wc_b")
            wv_b = bcast_row(wvor_d, "wv_b")
            bs_b = bcast_row(bsum_d, "bs_b") if has_bsum else None
            bc_b = bcast_row(bcent_d, "bc_b") if has_bcent else None
            bv_b = bcast_row(bvor_d, "bv_b") if has_bvor else None

            cent_sb = const.tile([P, NPAIR], F32, tag="cent_sb")
            nc.scalar.dma_start(out=cent_sb[:], in_=cent_d[:])
            vor_sb = const.tile([P, NPAIR], F32, tag="vor_sb")
            nc.scalar.dma_start(out=vor_sb[:], in_=vor_d[:])
            if has_bsum:
                odegraw_sb = const.tile([P, NPAIR], F32, tag="odegraw_sb")
                nc.scalar.dma_start(out=odegraw_sb[:], in_=odegraw_d[:])

            # --- main pipeline: pairs of 64-node chunks -------------------
            agg_tiles: dict[int, object] = {}
            ps_tiles: dict[int, list] = {}

            def phase1(cp):
                # one plain HWDGE DMA per pair; the sync ring carries ONLY
                # these so nothing stalls the stream
                W = int(tb[2 * cp + 2] - tb[2 * cp])
                at = apool.tile([P, WMAX * P], FP8)
                nc.sync.dma_start(out=at[:, :W * P], in_=a_ds[cp][:])
                ps_tiles[cp] = []
                for s in range(2):
                    k = 2 * cp + s
                    Tk = T[k]
                    off0 = int(tb[k] - tb[2 * cp])
                    ps = psp.tile([P, NW], F32, space="PSUM")
                    ps_tiles[cp].append(ps)
                    for t in range(Tk):
                        off = (off0 + t) * P
                        nc.tensor.matmul(
                            out=ps[:],
                            lhsT=at[:, off:off + P],
                            rhs=sel0_b[:],
                            start=(t == 0),
                            stop=(t == Tk - 1),
                        )

            def phase1_cast(cp):
                # lagged so DVE never queues behind PSUM waits
                aggt = aggp.tile([P, P], BF16)
                agg_tiles[cp] = aggt
                for s, ps in enumerate(ps_tiles.pop(cp)):
                    nc.vector.tensor_copy(aggt[:, s * NW:(s + 1) * NW], ps[:])

            def phase2(cp):
                aggt = agg_tiles.pop(cp)
                # node-major accumulation in PSUM:
                #   agg^T @ W_sum' + onehot_in^T @ z_in + onehot_out^T @ z_out
                pp = prp.tile([P, NODE_DIM], F32, space="PSUM")
                nsl = slice(cp * P, (cp + 1) * P)
                nc.tensor.matmul(out=pp[:], lhsT=aggt[:], rhs=wsum_b[:],
                                 start=True, stop=False, skip_group_check=True)
                for g in range(n_groups):
                    zi, k = zin_sb[g]
                    nc.tensor.matmul(out=pp[:], lhsT=ideg_oh[g][:k, nsl],
                                     rhs=zi[:k, :], start=False, stop=False,
                                     skip_group_check=True)
                    zo, k = zout_sb[g]
                    nc.tensor.matmul(out=pp[:], lhsT=odeg_oh[g][:k, nsl],
                                     rhs=zo[:k, :], start=False, stop=False,
                                     skip_group_check=True)
                # x lands in PSUM too: identity matmul over the pair's slice
                nc.tensor.matmul(out=pp[:], lhsT=ident_b[:],
                                 rhs=x_sb[:, cp * NODE_DIM:(cp + 1) * NODE_DIM],
                                 start=False, stop=True, skip_group_check=True)

                # sigmoid terms: one ACT op each (bias-free fast path)
                ct = tp.tile([P, NODE_DIM], F32, tag="ct")
                if has_bcent:
                    nc.vector.tensor_mul(
                        ct[:], wc_b[:],
                        cent_sb[:, cp:cp + 1].to_broadcast([P, NODE_DIM]))
                    nc.vector.tensor_add(ct[:], ct[:], bc_b[:])
                    nc.scalar.activation(out=ct[:], in_=ct[:], func=sig)
                else:
                    nc.scalar.activation(out=ct[:], in_=wc_b[:], func=sig,
                                         scale=cent_sb[:, cp:cp + 1])
                vt = tp.tile([P, NODE_DIM], F32, tag="vt")
                if has_bvor:
                    nc.vector.tensor_mul(
                        vt[:], wv_b[:],
                        vor_sb[:, cp:cp + 1].to_broadcast([P, NODE_DIM]))
                    nc.vector.tensor_add(vt[:], vt[:], bv_b[:])
                    nc.scalar.activation(out=vt[:], in_=vt[:], func=sig)
                else:
                    nc.scalar.activation(out=vt[:], in_=wv_b[:], func=sig,
                                         scale=vor_sb[:, cp:cp + 1])

                o = op.tile([P, NODE_DIM], BF16)
                # gpsimd on purpose: DVE is busier and Pool can read SBUF
                nc.gpsimd.tensor_add(o[:], ct[:], vt[:])
                osl = slice(cp * NODE_DIM, (cp + 1) * NODE_DIM)
                if has_bsum:
                    bst = tp.tile([P, NODE_DIM], F32, tag="bst")
                    nc.vector.tensor_mul(
                        bst[:], bs_b[:],
                        odegraw_sb[:, cp:cp + 1].to_broadcast([P, NODE_DIM]))
                    nc.vector.tensor_add(o[:], o[:], bst[:])
                nc.vector.tensor_add(o_sb[:, osl], o[:], pp[:])

            STCH = 10

            def phase3(cp):
                # batched store of finished column range, every STCH pairs
                if cp % STCH == STCH - 1 or cp == NPAIR - 1:
                    lo = (cp // STCH) * STCH * NODE_DIM
                    hi = (cp + 1) * NODE_DIM
                    nc.scalar.dma_start(out=out_d[:, lo:hi], in_=o_sb[:, lo:hi])

            CAST_LAG, LAG, SLAG = 2, 4, 7
            for cp in range(NPAIR + SLAG):
                if cp < NPAIR:
                    phase1(cp)
                if CAST_LAG <= cp < NPAIR + CAST_LAG:
                    phase1_cast(cp - CAST_LAG)
                if LAG <= cp < NPAIR + LAG:
                    phase2(cp - LAG)
                if cp >= SLAG:
                    phase3(cp - SLAG)

    return nc


# ----------------------------------------------------------------------------
def prepare_inputs(x, edge_index, edge_attr, voronoi_values, centralities,
                   z_in, z_out, W_cent, b_cent, W_vor, b_vor, W_sum, b_sum):
    """Host-side sharding: degree-sort nodes into 64-node chunks,
    snake-assign chunks to cores, place each node's edges into fixed
    (tile, partition) slots; quantize edge features to int8.
    Returns (in_maps, build_key, asm)."""
    src = np.asarray(edge_index[0], dtype=np.int64)
    dst = np.asarray(edge_index[1], dtype=np.int64)
    edge_attr = np.asarray(edge_attr, dtype=np.float32)
    x = np.asarray(x, dtype=np.float32)

    deg = np.bincount(src, minlength=N_NODES).astype(np.int64)
    in_deg_raw = np.bincount(dst, minlength=N_NODES).astype(np.int64)
    in_deg = np.minimum(in_deg_raw, MAX_DEG - 1).astype(np.int64)
    out_deg = np.minimum(deg, MAX_DEG - 1).astype(np.int64)
    n_groups = max(int(max(in_deg.max(), out_deg.max())) // P + 1, 1)

    # fp8 sigma-delta encoding scale; folds into W_sum
    sd = float(edge_attr[::17].std()) or 1.0
    step = sd / 32.0

    # degree-sorted chunking
    order_nodes = np.argsort(-deg, kind="stable")
    rank_of_node = np.empty(N_NODES, dtype=np.int64)
    rank_of_node[order_nodes] = np.arange(N_NODES)
    deg_rank = np.zeros(NRANK, dtype=np.int64)
    deg_rank[:N_NODES] = deg[order_nodes]

    T = np.maximum(1, (deg_rank[::8 * NW][:CH] + 1) // 2).astype(np.int64)
    tb = np.concatenate([[0], np.cumsum(T)])
    TOT = int(tb[-1])

    r_chunk = np.arange(N_CHUNKS)
    k_chunk = r_chunk // 8
    j_chunk = r_chunk % 8
    core_of_chunk = np.where(k_chunk % 2 == 0, j_chunk, 7 - j_chunk)

    # per-edge slot placement
    rho = rank_of_node[src]
    eorder = np.argsort(rho, kind="stable")
    rhos = rho[eorder]
    st_rank = np.concatenate([[0], np.cumsum(deg_rank)])[:-1]
    i_e = np.arange(N_EDGES, dtype=np.int64) - st_rank[rhos]
    rc = rhos // NW                      # chunk rank
    k_e = rc // 8                        # local chunk index (same per core)
    c_e = core_of_chunk[rc]
    p_e = (rhos % NW) + NW * (i_e % 2)
    g_e = tb[k_e] + i_e // 2
    flat = (c_e * P + p_e) * TOT + g_e

    # Sigma-delta fp8 encoding, per (node, feature) stream: q_i =
    # fp8(a_i + r_{i-1}), so the device's exact f32 sum of q telescopes to
    # segment_sum(a) minus one final fp8 residual per node — ~sqrt(deg)x
    # less error than independent rounding.  Encoding only: the device
    # still sums every edge's own vector.
    cs = (edge_attr[eorder] * (1.0 / step)).astype(np.float32)
    q8s = np.empty((N_EDGES, EDGE_DIM), dtype=FP8NP)
    resid = np.zeros((NRANK, EDGE_DIM), dtype=np.float32)
    maxd = int(deg_rank.max())
    for i in range(maxd):
        nodes_i = np.nonzero(deg_rank > i)[0]
        idx = st_rank[nodes_i] + i
        t = cs[idx] + resid[nodes_i]
        q = t.astype(FP8NP)
        q8s[idx] = q
        resid[nodes_i] = t - q.astype(np.float32)

    a_flat = np.zeros((N_CORES * P * TOT, EDGE_DIM), dtype=FP8NP)
    a_flat[flat] = q8s
    a_grid = a_flat.reshape(N_CORES, P, TOT * EDGE_DIM)
    # per-pair contiguous blocks (sequential HBM reads)
    a_pairs = [
        [np.ascontiguousarray(
            a_grid[c][:, int(tb[2 * cp]) * EDGE_DIM:int(tb[2 * cp + 2]) * EDGE_DIM])
         for cp in range(NPAIR)]
        for c in range(N_CORES)
    ]

    # node-rank -> (core, row) map
    rr = np.arange(NRANK)
    rc_r = rr // NW
    row_r = (rc_r // 8) * NW + rr % NW
    c_r = core_of_chunk[rc_r]
    rank_at = np.empty((N_CORES, NPAD), dtype=np.int64)
    rank_at[c_r, row_r] = rr

    def per_core(vals, dtype=None):
        # vals [N_NODES, ...] -> [N_CORES, NPAD, ...] via rank permutation
        shp = (NRANK,) + vals.shape[1:]
        padded = np.zeros(shp, dtype=vals.dtype)
        padded[:N_NODES] = vals[order_nodes]
        outv = padded[rank_at]
        return outv.astype(dtype) if dtype is not None else outv

    sel0 = (np.arange(P)[:, None] % NW == np.arange(NW)[None, :])
    sel0 = sel0.astype(FP8NP)

    # x in partition-major layout [P, NPAIR*NODE_DIM] (one contiguous DMA)
    x_p = per_core(x, FP8NP)
    x_pm = np.ascontiguousarray(
        x_p.reshape(N_CORES, NPAIR, P, NODE_DIM).transpose(0, 2, 1, 3)
        .reshape(N_CORES, P, NPAIR * NODE_DIM))
    # host-computed degree one-hots, [n_groups*P, NPAD] fp8 per core
    bins = np.arange(n_groups * P, dtype=np.int64)
    ideg_p = per_core(in_deg[:, None])[..., 0]
    odeg_p = per_core(out_deg[:, None])[..., 0]
    ohi = (ideg_p[:, None, :] == bins[None, :, None]).astype(FP8NP)
    oho = (odeg_p[:, None, :] == bins[None, :, None]).astype(FP8NP)
    odegraw_p = per_core(deg.astype(np.float32)[:, None])
    cent_p = per_core(np.asarray(centralities, dtype=np.float32))
    vor_p = per_core(np.asarray(voronoi_values, dtype=np.float32))

    def col_layout(a):  # [NPAD,1] -> [P, NPAIR]  (partition-major)
        return np.ascontiguousarray(a.reshape(NPAIR, P).T)

    z_in = np.asarray(z_in, dtype=np.float32).astype(FP8NP)
    z_out = np.asarray(z_out, dtype=np.float32).astype(FP8NP)
    W_sum_eff = np.asarray(W_sum, dtype=np.float32) * step
    row = lambda v: np.ascontiguousarray(
        np.asarray(v, dtype=np.float32).reshape(1, NODE_DIM))
    b_sum_r, b_cent_r, b_vor_r = row(b_sum), row(b_cent), row(b_vor)
    flags = (bool(np.any(b_sum_r)), bool(np.any(b_cent_r)),
             bool(np.any(b_vor_r)))

    in_maps = []
    for c in range(N_CORES):
        in_maps.append({
            **{f"a{cp}": a_pairs[c][cp] for cp in range(NPAIR)},
            "sel0": sel0,
            "x": x_pm[c],
            "ident": np.eye(P, dtype=np.float32).astype(FP8NP),
            "ohi": np.ascontiguousarray(ohi[c]),
            "oho": np.ascontiguousarray(oho[c]),
            "odegraw": col_layout(odegraw_p[c]),
            "cent": col_layout(cent_p[c]),
            "vor": col_layout(vor_p[c]),
            "z_in": z_in,
            "z_out": z_out,
            "W_sum": W_sum_eff,
            "b_sum": b_sum_r,
            "W_cent": row(W_cent),
            "b_cent": b_cent_r,
            "W_vor": row(W_vor),
            "b_vor": b_vor_r,
        })
    asm = {"order_nodes": order_nodes, "c_r": c_r, "row_r": row_r}
    key = (tuple(int(t) for t in T), n_groups) + flags
    return in_maps, key, asm


def assemble_output(results, asm):
    """results: list of per-core 'out' arrays [P, NPAIR*NODE_DIM]."""
    outs = np.stack([
        np.asarray(results[c], dtype=np.float32)
        .reshape(P, NPAIR, NODE_DIM).transpose(1, 0, 2).reshape(NPAD, NODE_DIM)
        for c in range(N_CORES)
    ])
    out_sorted = outs[asm["c_r"], asm["row_r"]]       # [NRANK, 256]
    out_full = np.empty((N_NODES, NODE_DIM), dtype=np.float32)
    out_full[asm["order_nodes"]] = out_sorted[:N_NODES]
    return out_full


_PROGRAM_CACHE: dict[tuple, bass.Bass] = {}


def kernel(**inputs) -> np.ndarray:
    in_maps, key, asm = prepare_inputs(**inputs)
    nc = _PROGRAM_CACHE.get(key)
    if nc is None:
        nc = build_program(*key)
        _PROGRAM_CACHE[key] = nc
    res = None
    for attempt in range(3):
        try:
            res = run_bass_kernel_spmd(nc, in_maps, core_ids=list(range(N_CORES)))
            break
        except Exception:
            # axon transiently reports "accelerator device unrecoverable";
            # a clean retry succeeds
            if attempt == 2:
                raise
    return assemble_output([res.results[i]["out"] for i in range(N_CORES)], asm)
